# revision 20
# baseline (speedup 1.0000x reference)
"""Bass/Trainium2 kernel for nn_DotProductAttention_47528108097846.

reference:
    scores = einsum('bhqd,bhkd->bhqk', Q, K) / 16
    attn = softmax(scores, axis=-1)
    h = einsum('bhqk,bhkd->bhqd', attn, V)
    return reshape(h, (S, B, H, D))

B=2, H=8, S=4096, D=64. 16 (b,h) pairs sharded as 2 per NeuronCore across 8
cores (batch+head parallel, no cross-core comms).

Per-core design (2 heads A/B):
  - QT/KT stacked: head A's transposed Q/K (d=64 contraction rows) on SBUF
    partitions 0:64, head B's on 64:128. Per k-block the two heads' QK
    matmuls are disjoint row-group tiles (contraction 64 each) that the PE
    runs concurrently -> 2x QK throughput vs a 128-padded contraction. Head A
    scores land in cols 0:512, head B in 512:1024 of one [128, 1024] fp32
    PSUM slot, double buffered.
  - exp per k-block on the packed [128, 1024] slot: ScalarE activation
    (scale=1/16 fused); every DVE_EVERY-th k-block offloaded to VectorE as a
    Schraudolph bit-trick exp (bf16 bits = int16(s*128/(16 ln2) +
    128*(127-0.0573)), one tensor_scalar). ~2% rms error on the offloaded
    share only -> net rel err ~1.1e-2 (gate 2e-2).
  - AV per head: lhsT = V' = [V | 1 | 0pad] ([128, 128] bf16, FWL-eligible;
    ones column accumulates the softmax denominator in output row 64),
    accumulating [65, 512] fp32 PSUM.
  - Prologue is streamed: K/V/Q arrive in chunks interleaved with the first
    q-group's k-loop so the PE's in-order queue never parks behind the
    transposes. Per-q-group epilogues (PE transpose + reciprocal scale) are
    deferred into the next q-group's k-loop.
"""
import math

import numpy as np

import concourse.bass as bass
import concourse.bacc as bacc
import concourse.tile as tile
from concourse import mybir
from concourse.masks import make_identity
from concourse.bass_utils import run_bass_kernel_spmd

B, H, S, D = 2, 8, 4096, 64
N_CORES = 8
PAIRS_PER_CORE = (B * H) // N_CORES  # 2 heads per core

f32 = mybir.dt.float32
bf16 = mybir.dt.bfloat16
i16 = mybir.dt.int16

QG = 512             # q-group width (per-head scores = QG*4B = 1 PSUM bank)
NQG = S // QG        # 8 q-groups
NKB = S // 128       # 32 k-blocks (128 k-positions each)
CH = 4               # transpose chunk: CH*128 q/k columns per chunk
NCHUNK = NKB // CH   # 8 chunks per tensor

DVE_EVERY = 2        # every 2nd k-block's exp goes to VectorE (bit-trick)
SCH_A = 128.0 / (16.0 * math.log(2.0))
SCH_B = 128.0 * (127.0 - 0.0573)


def build_attention(nc, tc, q, k, v, o):
    import contextlib
    ctx = contextlib.ExitStack()
    consts = ctx.enter_context(tc.tile_pool(name="consts", bufs=1))
    nat = ctx.enter_context(tc.tile_pool(name="nat", bufs=2))
    persist = ctx.enter_context(tc.tile_pool(name="persist", bufs=1))
    sb = ctx.enter_context(tc.tile_pool(name="sb", bufs=2))
    pool_e = ctx.enter_context(tc.tile_pool(name="sb_e", bufs=4))
    pool_s = ctx.enter_context(tc.tile_pool(name="ps_s", bufs=2, space="PSUM"))
    pool_o = ctx.enter_context(tc.tile_pool(name="ps_o", bufs=1, space="PSUM"))
    pool_t = ctx.enter_context(tc.tile_pool(name="ps_t", bufs=2, space="PSUM"))

    ident = consts.tile([128, 128], f32)
    make_identity(nc, ident)
    identb = consts.tile([128, 128], bf16)
    nc.vector.tensor_copy(out=identb, in_=ident)

    # HAM warmup: ~3.5us of dummy matmuls at t=0 so the PE clock-gate
    # un-throttles (1.2 -> 2.4 GHz) before the real work arrives, instead of
    # ~20us into the kernel. Reads an uninitialized const tile (values are
    # irrelevant), writes a scratch PSUM slot.
    warm_src = consts.tile([128, 256], bf16, tag="warm")
    nc.gpsimd.memset(warm_src, 0.0)
    for _ in range(16):
        ps_warm = pool_t.tile([128, CH * 128], bf16, tag="t")
        nc.tensor.matmul(out=ps_warm.bitcast(f32)[:, 0:256], lhsT=identb,
                         rhs=warm_src, start=True, stop=True)

    # persistent stacked QT/KT: partitions 0:64 head A (d rows), 64:128 head B
    qt = persist.tile([128, NKB, 128], bf16, tag="qt")
    kt = persist.tile([128, NKB, 128], bf16, tag="kt")
    v1s = []
    for h in range(PAIRS_PER_CORE):
        v1 = persist.tile([128, NKB, 65], bf16, tag=f"v1{h}")
        v1s.append(v1)
        nc.gpsimd.memset(v1[:, :, 64:65], 1.0)

    def emit_vchunk(c, eng=None):
        # V rows for k-blocks [8c, 8c+8) of both heads
        nv = NKB // 4
        for h in range(PAIRS_PER_CORE):
            vnat = nat.tile([128, nv, 64], f32, tag="vnat")
            nc.sync.dma_start(
                out=vnat,
                in_=v[h].rearrange("(n p) d -> p n d", p=128)[
                    :, c * nv:(c + 1) * nv, :])
            (eng or nc.vector).tensor_copy(
                out=v1s[h][:, c * nv:(c + 1) * nv, 0:64],
                in_=vnat)

    def emit_chunk_load(src, g, eng=None):
        # DMA + cast chunk g (CH*128 rows) of both heads into one
        # [128, CH, 128] bf16 tile (head A cols 0:64, head B cols 64:128)
        natbc = nat.tile([128, CH, 128], bf16, tag="natb")
        for h in range(PAIRS_PER_CORE):
            natc = nat.tile([128, CH, 64], f32, tag=f"nat{h}")
            nc.sync.dma_start(
                out=natc,
                in_=src[h].rearrange("(n p) d -> p n d", p=128)[
                    :, g * CH:(g + 1) * CH, :])
            (eng or nc.vector).tensor_copy(
                out=natbc[:, :, h * 64:(h + 1) * 64], in_=natc)
        ps_tr = pool_t.tile([128, CH * 128], bf16, tag="t")
        return natbc, ps_tr.rearrange("p (a b) -> p a b", a=CH)

    def emit_chunk_tr(state, js):
        natbc, tr4 = state
        for j in js:
            nc.tensor.transpose(tr4[:, j, :], natbc[:, j, :], identb)

    def emit_chunk_copy(state, dst, g):
        nc.vector.tensor_copy(
            out=dst[:, g * CH:(g + 1) * CH, :], in_=state[1])

    def emit_chunk(src, dst, g, eng=None):
        st = emit_chunk_load(src, g, eng)
        emit_chunk_tr(st, range(CH))
        emit_chunk_copy(st, dst, g)

    emit_chunk(k, kt, 0, eng=nc.vector)
    emit_chunk(q, qt, 0, eng=nc.vector)
    emit_vchunk(0, eng=nc.vector)
    emit_chunk(k, kt, 1, eng=nc.vector)

    qt_f = qt.rearrange("p n d -> p (n d)")  # [128, S] q columns

    pending_epi = []
    epi_state = {}

    def emit_epi_strip(qg, oTs, strip):
        # one output strip: PE transpose + reciprocal-scale; DMA per head
        # after its 4th strip. Called with strip=0..7 spread across k-blocks.
        h, i = divmod(strip, QG // 128)
        oT = oTs[h]
        if i == 0:
            epi_state[h] = sb.tile([128, QG // 128, 64], f32, tag=f"out{h}",
                                   name=f"outsb{h}")
        out_sb = epi_state[h]
        ps_tr = pool_t.tile([128, CH * 128], bf16, tag="t")
        ps_t = ps_tr[:, 0:65]
        nc.tensor.transpose(
            ps_t, oT[:, i * 128:(i + 1) * 128], identb[0:65, 0:65])
        rcp = sb.tile([128, 1], f32, tag="rcp")
        nc.vector.reciprocal(out=rcp, in_=ps_t[:, 64:65])
        nc.scalar.mul(out=out_sb[:, i, :], in_=ps_t[:, 0:64], mul=rcp)
        if i == QG // 128 - 1:
            out_r = o[h].rearrange("(n p) d -> p n d", p=128)
            nc.sync.dma_start(
                out=out_r[:, qg * (QG // 128):(qg + 1) * (QG // 128), :],
                in_=out_sb)

    def emit_epilogue(qg, oTs):
        for strip in range(2 * (QG // 128)):
            emit_epi_strip(qg, oTs, strip)

    # ---------------- main loop --------------------------------------
    for qg in range(NQG):
        ps_oA = pool_o.tile([65, QG], f32, tag="oA")
        ps_oB = pool_o.tile([65, QG], f32, tag="oB")

        def av(prev_eT, prev_kb):
            nc.tensor.matmul(
                out=ps_oA, lhsT=v1s[0][:, prev_kb, :],
                rhs=prev_eT[:, 0:QG],
                start=(prev_kb == 0), stop=(prev_kb == NKB - 1))
            nc.tensor.matmul(
                out=ps_oB, lhsT=v1s[1][:, prev_kb, :],
                rhs=prev_eT[:, QG:2 * QG],
                start=(prev_kb == 0), stop=(prev_kb == NKB - 1))

        prev = []
        kst = qst = None
        for kb in range(NKB):
            if qg == 0:
                # stream the rest of K/V in, spread so the in-order PE queue
                # never gets a transpose burst ahead of a QK pair
                c = kb // CH + 2
                ph = kb % CH
                if c < NCHUNK:
                    if ph == 0:
                        kst = emit_chunk_load(k, c)
                        emit_chunk_tr(kst, (0, 1))
                    elif ph == 1:
                        emit_chunk_tr(kst, (2, 3))
                        emit_chunk_copy(kst, kt, c)
                if kb in (2, 10, 18):
                    emit_vchunk(kb // 8 + 1)
            if qg + 1 < NQG:
                if kb == 20:
                    qst = emit_chunk_load(q, qg + 1)
                    emit_chunk_tr(qst, (0, 1))
                elif kb == 22:
                    emit_chunk_tr(qst, (2, 3))
                    emit_chunk_copy(qst, qt, qg + 1)
            if pending_epi and 4 <= kb <= 18 and kb % 2 == 0:
                emit_epi_strip(*pending_epi[0], strip=(kb - 4) // 2)
                if kb == 18:
                    pending_epi.pop()
            ps = pool_s.tile([128, 2 * QG], f32, tag="s")
            nc.tensor.matmul(
                out=ps[:, 0:QG], lhsT=kt[0:64, kb, :],
                rhs=qt_f[0:64, qg * QG:(qg + 1) * QG],
                start=True, stop=True)
            nc.tensor.matmul(
                out=ps[:, QG:2 * QG], lhsT=kt[64:128, kb, :],
                rhs=qt_f[64:128, qg * QG:(qg + 1) * QG],
                start=True, stop=True)
            eT = pool_e.tile([128, 2 * QG], bf16, tag="exp")
            if kb % DVE_EVERY == DVE_EVERY - 1:
                nc.vector.tensor_scalar(
                    out=eT.bitcast(i16), in0=ps,
                    scalar1=SCH_A, scalar2=SCH_B,
                    op0=mybir.AluOpType.mult, op1=mybir.AluOpType.add)
            else:
                nc.scalar.activation(
                    out=eT, in_=ps,
                    func=mybir.ActivationFunctionType.Exp,
                    scale=1.0 / 16.0)
            # depth-2 software pipeline: AV for kb-2 is emitted after QK(kb)
            # so the in-order PE queue always has the next QK ahead of the
            # exp-dependent AVs -> ScalarE/DVE never starve.
            prev.append((eT, kb))
            if len(prev) > 2:
                av(*prev.pop(0))
        for pe_ in prev:
            av(*pe_)

        # drain ps_o to SBUF now (frees the banks for the next q-group);
        # the PE-transpose + normalize part is deferred into the next
        # q-group's k-loop.
        oTs = []
        for h, ps_oX in ((0, ps_oA), (1, ps_oB)):
            oT = sb.tile([65, QG], bf16, tag=f"oT{h}")
            nc.scalar.copy(out=oT, in_=ps_oX)
            oTs.append(oT)
        pending_epi.append((qg, oTs))

    emit_epilogue(*pending_epi.pop())
    ctx.close()


_CACHED = {}


def build_program():
    key = "default"
    if key in _CACHED:
        return _CACHED[key]
    nc = bacc.Bacc("TRN2", target_bir_lowering=False, debug=False,
                   num_devices=N_CORES)
    q = nc.dram_tensor("q", [PAIRS_PER_CORE, S, D], f32,
                       kind="ExternalInput").ap()
    k = nc.dram_tensor("k", [PAIRS_PER_CORE, S, D], f32,
                       kind="ExternalInput").ap()
    v = nc.dram_tensor("v", [PAIRS_PER_CORE, S, D], f32,
                       kind="ExternalInput").ap()
    o = nc.dram_tensor("o", [PAIRS_PER_CORE, S, D], f32,
                       kind="ExternalOutput").ap()
    with tile.TileContext(nc) as tc:
        build_attention(nc, tc, q, k, v, o)
    nc.compile()
    _CACHED[key] = nc
    return nc


def kernel(queries, keys, values, adj=None, **_unused):
    """Full-input attention on 8 NeuronCores. Returns [S, B, H, D] fp32."""
    queries = np.ascontiguousarray(queries, dtype=np.float32)
    keys = np.ascontiguousarray(keys, dtype=np.float32)
    values = np.ascontiguousarray(values, dtype=np.float32)

    nc = build_program()
    qf = queries.reshape(B * H, S, D)
    kf = keys.reshape(B * H, S, D)
    vf = values.reshape(B * H, S, D)
    in_maps = []
    for c in range(N_CORES):
        sl = slice(c * PAIRS_PER_CORE, (c + 1) * PAIRS_PER_CORE)
        in_maps.append({"q": qf[sl], "k": kf[sl], "v": vf[sl]})
    res = run_bass_kernel_spmd(nc, in_maps, list(range(N_CORES)))
    hout = np.empty((B * H, S, D), dtype=np.float32)
    for c in range(N_CORES):
        hout[c * PAIRS_PER_CORE:(c + 1) * PAIRS_PER_CORE] = res.results[c]["o"]
    return hout.reshape(B, H, S, D).reshape(S, B, H, D)


# revision 23
# speedup vs baseline: 1.0079x; 1.0079x over previous
"""Bass/Trainium2 kernel for nn_DotProductAttention_47528108097846.

reference:
    scores = einsum('bhqd,bhkd->bhqk', Q, K) / 16
    attn = softmax(scores, axis=-1)
    h = einsum('bhqk,bhkd->bhqd', attn, V)
    return reshape(h, (S, B, H, D))

B=2, H=8, S=4096, D=64. 16 (b,h) pairs sharded as 2 per NeuronCore across 8
cores (batch+head parallel, no cross-core comms).

Per-core design (2 heads A/B):
  - QT/KT stacked: head A's transposed Q/K (d=64 contraction rows) on SBUF
    partitions 0:64, head B's on 64:128. Per k-block the two heads' QK
    matmuls are disjoint row-group tiles (contraction 64 each) that the PE
    runs concurrently -> 2x QK throughput vs a 128-padded contraction. Head A
    scores land in cols 0:512, head B in 512:1024 of one [128, 1024] fp32
    PSUM slot, double buffered.
  - exp per k-block on the packed [128, 1024] slot: ScalarE activation
    (scale=1/16 fused); every DVE_EVERY-th k-block offloaded to VectorE as a
    Schraudolph bit-trick exp (bf16 bits = int16(s*128/(16 ln2) +
    128*(127-0.0573)), one tensor_scalar). ~2% rms error on the offloaded
    share only -> net rel err ~1.1e-2 (gate 2e-2).
  - AV per head: lhsT = V' = [V | 1 | 0pad] ([128, 128] bf16, FWL-eligible;
    ones column accumulates the softmax denominator in output row 64),
    accumulating [65, 512] fp32 PSUM.
  - Prologue is streamed: K/V/Q arrive in chunks interleaved with the first
    q-group's k-loop so the PE's in-order queue never parks behind the
    transposes. Per-q-group epilogues (PE transpose + reciprocal scale) are
    deferred into the next q-group's k-loop.
"""
import math

import numpy as np

import concourse.bass as bass
import concourse.bacc as bacc
import concourse.tile as tile
from concourse import mybir
from concourse.masks import make_identity
from concourse.bass_utils import run_bass_kernel_spmd

B, H, S, D = 2, 8, 4096, 64
N_CORES = 8
PAIRS_PER_CORE = (B * H) // N_CORES  # 2 heads per core

f32 = mybir.dt.float32
bf16 = mybir.dt.bfloat16
i16 = mybir.dt.int16

QG = 512             # q-group width (per-head scores = QG*4B = 1 PSUM bank)
NQG = S // QG        # 8 q-groups
NKB = S // 128       # 32 k-blocks (128 k-positions each)
CH = 4               # transpose chunk: CH*128 q/k columns per chunk
NCHUNK = NKB // CH   # 8 chunks per tensor

DVE_EVERY = 2        # every 2nd k-block's exp goes to VectorE (bit-trick)
SCH_A = 128.0 / (16.0 * math.log(2.0))
SCH_B = 128.0 * (127.0 - 0.0573)


def build_attention(nc, tc, q, k, v, o):
    import contextlib
    ctx = contextlib.ExitStack()
    consts = ctx.enter_context(tc.tile_pool(name="consts", bufs=1))
    nat = ctx.enter_context(tc.tile_pool(name="nat", bufs=2))
    persist = ctx.enter_context(tc.tile_pool(name="persist", bufs=1))
    sb = ctx.enter_context(tc.tile_pool(name="sb", bufs=2))
    pool_e = ctx.enter_context(tc.tile_pool(name="sb_e", bufs=4))
    pool_s = ctx.enter_context(tc.tile_pool(name="ps_s", bufs=2, space="PSUM"))
    pool_o = ctx.enter_context(tc.tile_pool(name="ps_o", bufs=1, space="PSUM"))
    pool_t = ctx.enter_context(tc.tile_pool(name="ps_t", bufs=2, space="PSUM"))

    ident = consts.tile([128, 128], f32)
    make_identity(nc, ident)
    identb = consts.tile([128, 128], bf16)
    nc.vector.tensor_copy(out=identb, in_=ident)

    # HAM warmup: ~3.5us of dummy matmuls at t=0 so the PE clock-gate
    # un-throttles (1.2 -> 2.4 GHz) before the real work arrives, instead of
    # ~20us into the kernel. Reads an uninitialized const tile (values are
    # irrelevant), writes a scratch PSUM slot.
    warm_src = consts.tile([128, 256], bf16, tag="warm")
    nc.gpsimd.memset(warm_src, 0.0)
    for _ in range(16):
        ps_warm = pool_t.tile([128, CH * 128], bf16, tag="t")
        nc.tensor.matmul(out=ps_warm.bitcast(f32)[:, 0:256], lhsT=identb,
                         rhs=warm_src, start=True, stop=True)

    # persistent stacked QT/KT: partitions 0:64 head A (d rows), 64:128 head B
    qt = persist.tile([128, NKB, 128], bf16, tag="qt")
    kt = persist.tile([128, NKB, 128], bf16, tag="kt")
    v1s = []
    for h in range(PAIRS_PER_CORE):
        v1 = persist.tile([128, NKB, 65], bf16, tag=f"v1{h}")
        v1s.append(v1)
        nc.gpsimd.memset(v1[:, :, 64:65], 1.0)

    def emit_vchunk(c, eng=None):
        # V rows for k-blocks [8c, 8c+8) of both heads
        nv = NKB // 4
        for h in range(PAIRS_PER_CORE):
            vnat = nat.tile([128, nv, 64], f32, tag="vnat")
            nc.sync.dma_start(
                out=vnat,
                in_=v[h].rearrange("(n p) d -> p n d", p=128)[
                    :, c * nv:(c + 1) * nv, :])
            (eng or nc.vector).tensor_copy(
                out=v1s[h][:, c * nv:(c + 1) * nv, 0:64],
                in_=vnat)

    def emit_chunk_load(src, g, eng=None):
        # DMA + cast chunk g (CH*128 rows) of both heads into one
        # [128, CH, 128] bf16 tile (head A cols 0:64, head B cols 64:128)
        natbc = nat.tile([128, CH, 128], bf16, tag="natb")
        for h in range(PAIRS_PER_CORE):
            natc = nat.tile([128, CH, 64], f32, tag=f"nat{h}")
            nc.sync.dma_start(
                out=natc,
                in_=src[h].rearrange("(n p) d -> p n d", p=128)[
                    :, g * CH:(g + 1) * CH, :])
            (eng or nc.vector).tensor_copy(
                out=natbc[:, :, h * 64:(h + 1) * 64], in_=natc)
        ps_tr = pool_t.tile([128, CH * 128], bf16, tag="t")
        return natbc, ps_tr.rearrange("p (a b) -> p a b", a=CH)

    def emit_chunk_tr(state, js):
        natbc, tr4 = state
        for j in js:
            nc.tensor.transpose(tr4[:, j, :], natbc[:, j, :], identb)

    def emit_chunk_copy(state, dst, g):
        nc.vector.tensor_copy(
            out=dst[:, g * CH:(g + 1) * CH, :], in_=state[1])

    def emit_chunk(src, dst, g, eng=None):
        st = emit_chunk_load(src, g, eng)
        emit_chunk_tr(st, range(CH))
        emit_chunk_copy(st, dst, g)

    emit_chunk(k, kt, 0, eng=nc.vector)
    emit_chunk(q, qt, 0, eng=nc.vector)
    emit_vchunk(0, eng=nc.vector)
    emit_chunk(k, kt, 1, eng=nc.vector)

    qt_f = qt.rearrange("p n d -> p (n d)")  # [128, S] q columns

    pending_epi = []
    epi_state = {}

    def emit_epi_strip(qg, oTs, strip):
        # one output strip: PE transpose + reciprocal-scale; DMA per head
        # after its 4th strip. Called with strip=0..7 spread across k-blocks.
        h, i = divmod(strip, QG // 128)
        oT = oTs[h]
        if i == 0:
            epi_state[h] = sb.tile([128, QG // 128, 64], f32, tag=f"out{h}",
                                   name=f"outsb{h}")
        out_sb = epi_state[h]
        ps_tr = pool_t.tile([128, CH * 128], bf16, tag="t")
        ps_t = ps_tr[:, 0:65]
        nc.tensor.transpose(
            ps_t, oT[:, i * 128:(i + 1) * 128], identb[0:65, 0:65])
        rcp = sb.tile([128, 1], f32, tag="rcp")
        nc.vector.reciprocal(out=rcp, in_=ps_t[:, 64:65])
        nc.scalar.mul(out=out_sb[:, i, :], in_=ps_t[:, 0:64], mul=rcp)
        if i == QG // 128 - 1:
            out_r = o[h].rearrange("(n p) d -> p n d", p=128)
            nc.sync.dma_start(
                out=out_r[:, qg * (QG // 128):(qg + 1) * (QG // 128), :],
                in_=out_sb)

    def emit_epilogue(qg, oTs):
        for strip in range(2 * (QG // 128)):
            emit_epi_strip(qg, oTs, strip)

    # ---------------- main loop --------------------------------------
    for qg in range(NQG):
        ps_oA = pool_o.tile([65, QG], f32, tag="oA")
        ps_oB = pool_o.tile([65, QG], f32, tag="oB")

        def av(prev_eT, prev_kb):
            nc.tensor.matmul(
                out=ps_oA, lhsT=v1s[0][:, prev_kb, :],
                rhs=prev_eT[:, 0:QG],
                start=(prev_kb == 0), stop=(prev_kb == NKB - 1))
            nc.tensor.matmul(
                out=ps_oB, lhsT=v1s[1][:, prev_kb, :],
                rhs=prev_eT[:, QG:2 * QG],
                start=(prev_kb == 0), stop=(prev_kb == NKB - 1))

        prev = []
        kst = qst = None
        for kb in range(NKB):
            if qg == 0:
                # stream the rest of K/V in, spread so the in-order PE queue
                # never gets a transpose burst ahead of a QK pair
                c = kb // CH + 2
                ph = kb % CH
                if c < NCHUNK:
                    if ph == 0:
                        kst = emit_chunk_load(k, c)
                        emit_chunk_tr(kst, (0, 1))
                    elif ph == 1:
                        emit_chunk_tr(kst, (2, 3))
                        emit_chunk_copy(kst, kt, c)
                if kb in (2, 10, 18):
                    emit_vchunk(kb // 8 + 1)
            if qg + 1 < NQG:
                if kb == 20:
                    qst = emit_chunk_load(q, qg + 1)
                    emit_chunk_tr(qst, (0, 1))
                elif kb == 22:
                    emit_chunk_tr(qst, (2, 3))
                    emit_chunk_copy(qst, qt, qg + 1)
            if pending_epi and 4 <= kb <= 18 and kb % 2 == 0:
                emit_epi_strip(*pending_epi[0], strip=(kb - 4) // 2)
                if kb == 18:
                    pending_epi.pop()
            ps = pool_s.tile([128, 2 * QG], f32, tag="s")
            nc.tensor.matmul(
                out=ps[:, 0:QG], lhsT=kt[0:64, kb, :],
                rhs=qt_f[0:64, qg * QG:(qg + 1) * QG],
                start=True, stop=True)
            nc.tensor.matmul(
                out=ps[:, QG:2 * QG], lhsT=kt[64:128, kb, :],
                rhs=qt_f[64:128, qg * QG:(qg + 1) * QG],
                start=True, stop=True)
            eT = pool_e.tile([128, 2 * QG], bf16, tag="exp")
            if kb % DVE_EVERY == DVE_EVERY - 1:
                nc.vector.tensor_scalar(
                    out=eT.bitcast(i16), in0=ps,
                    scalar1=SCH_A, scalar2=SCH_B,
                    op0=mybir.AluOpType.mult, op1=mybir.AluOpType.add)
            else:
                nc.scalar.activation(
                    out=eT, in_=ps,
                    func=mybir.ActivationFunctionType.Exp,
                    scale=1.0 / 16.0)
            # depth-2 software pipeline: AV for kb-2 is emitted after QK(kb)
            # so the in-order PE queue always has the next QK ahead of the
            # exp-dependent AVs -> ScalarE/DVE never starve.
            prev.append((eT, kb))
            if len(prev) > 2:
                av(*prev.pop(0))
        for pe_ in prev:
            av(*pe_)

        # drain ps_o to SBUF now (frees the banks for the next q-group);
        # the PE-transpose + normalize part is deferred into the next
        # q-group's k-loop.
        oTs = []
        for h, ps_oX in ((0, ps_oA), (1, ps_oB)):
            oT = sb.tile([65, QG], bf16, tag=f"oT{h}")
            nc.scalar.copy(out=oT, in_=ps_oX)
            oTs.append(oT)
        pending_epi.append((qg, oTs))

    emit_epilogue(*pending_epi.pop())
    ctx.close()


_CACHED = {}


def build_program():
    key = "default"
    if key in _CACHED:
        return _CACHED[key]
    nc = bacc.Bacc("TRN2", target_bir_lowering=False, debug=False,
                   num_devices=N_CORES)
    q = nc.dram_tensor("q", [PAIRS_PER_CORE, S, D], f32,
                       kind="ExternalInput").ap()
    k = nc.dram_tensor("k", [PAIRS_PER_CORE, S, D], f32,
                       kind="ExternalInput").ap()
    v = nc.dram_tensor("v", [PAIRS_PER_CORE, S, D], f32,
                       kind="ExternalInput").ap()
    o = nc.dram_tensor("o", [PAIRS_PER_CORE, S, D], f32,
                       kind="ExternalOutput").ap()
    with tile.TileContext(nc) as tc:
        build_attention(nc, tc, q, k, v, o)
    nc.compile()
    _CACHED[key] = nc
    return nc


def kernel(queries, keys, values, adj=None, **_unused):
    """Full-input attention on 8 NeuronCores. Returns [S, B, H, D] fp32."""
    queries = np.ascontiguousarray(queries, dtype=np.float32)
    keys = np.ascontiguousarray(keys, dtype=np.float32)
    values = np.ascontiguousarray(values, dtype=np.float32)

    nc = build_program()
    qf = queries.reshape(B * H, S, D)
    kf = keys.reshape(B * H, S, D)
    vf = values.reshape(B * H, S, D)
    in_maps = []
    for c in range(N_CORES):
        sl = slice(c * PAIRS_PER_CORE, (c + 1) * PAIRS_PER_CORE)
        in_maps.append({"q": qf[sl], "k": kf[sl], "v": vf[sl]})
    res = run_bass_kernel_spmd(nc, in_maps, list(range(N_CORES)))
    hout = np.empty((B * H, S, D), dtype=np.float32)
    for c in range(N_CORES):
        hout[c * PAIRS_PER_CORE:(c + 1) * PAIRS_PER_CORE] = res.results[c]["o"]
    return hout.reshape(B, H, S, D).reshape(S, B, H, D)


# revision 28
# speedup vs baseline: 1.0089x; 1.0010x over previous
"""Bass/Trainium2 kernel for nn_DotProductAttention_47528108097846.

reference:
    scores = einsum('bhqd,bhkd->bhqk', Q, K) / 16
    attn = softmax(scores, axis=-1)
    h = einsum('bhqk,bhkd->bhqd', attn, V)
    return reshape(h, (S, B, H, D))

B=2, H=8, S=4096, D=64. 16 (b,h) pairs sharded as 2 per NeuronCore across 8
cores (batch+head parallel, no cross-core comms).

Per-core design (2 heads A/B):
  - QT/KT stacked: head A's transposed Q/K (d=64 contraction rows) on SBUF
    partitions 0:64, head B's on 64:128. Per k-block the two heads' QK
    matmuls are disjoint row-group tiles (contraction 64 each) that the PE
    runs concurrently -> 2x QK throughput vs a 128-padded contraction. Head A
    scores land in cols 0:512, head B in 512:1024 of one [128, 1024] fp32
    PSUM slot, double buffered.
  - exp per k-block on the packed [128, 1024] slot: ScalarE activation
    (scale=1/16 fused); every DVE_EVERY-th k-block offloaded to VectorE as a
    Schraudolph bit-trick exp (bf16 bits = int16(s*128/(16 ln2) +
    128*(127-0.0573)), one tensor_scalar). ~2% rms error on the offloaded
    share only -> net rel err ~1.1e-2 (gate 2e-2).
  - AV per head: lhsT = V' = [V | 1 | 0pad] ([128, 128] bf16, FWL-eligible;
    ones column accumulates the softmax denominator in output row 64),
    accumulating [65, 512] fp32 PSUM.
  - Prologue is streamed: K/V/Q arrive in chunks interleaved with the first
    q-group's k-loop so the PE's in-order queue never parks behind the
    transposes. Per-q-group epilogues (PE transpose + reciprocal scale) are
    deferred into the next q-group's k-loop.
"""
import math

import numpy as np

import concourse.bass as bass
import concourse.bacc as bacc
import concourse.tile as tile
from concourse import mybir
from concourse.masks import make_identity
from concourse.bass_utils import run_bass_kernel_spmd

B, H, S, D = 2, 8, 4096, 64
N_CORES = 8
PAIRS_PER_CORE = (B * H) // N_CORES  # 2 heads per core

f32 = mybir.dt.float32
bf16 = mybir.dt.bfloat16
i16 = mybir.dt.int16

QG = 512             # q-group width (per-head scores = QG*4B = 1 PSUM bank)
NQG = S // QG        # 8 q-groups
NKB = S // 128       # 32 k-blocks (128 k-positions each)
CH = 4               # transpose chunk: CH*128 q/k columns per chunk
NCHUNK = NKB // CH   # 8 chunks per tensor

DVE_EVERY = 2        # every 2nd k-block's exp goes to VectorE (bit-trick)
SCH_A = 128.0 / (16.0 * math.log(2.0))
SCH_B = 128.0 * (127.0 - 0.0573)


def build_attention(nc, tc, q, k, v, o):
    import contextlib
    ctx = contextlib.ExitStack()
    consts = ctx.enter_context(tc.tile_pool(name="consts", bufs=1))
    nat = ctx.enter_context(tc.tile_pool(name="nat", bufs=2))
    persist = ctx.enter_context(tc.tile_pool(name="persist", bufs=1))
    sb = ctx.enter_context(tc.tile_pool(name="sb", bufs=2))
    pool_e = ctx.enter_context(tc.tile_pool(name="sb_e", bufs=4))
    pool_s = ctx.enter_context(tc.tile_pool(name="ps_s", bufs=2, space="PSUM"))
    pool_o = ctx.enter_context(tc.tile_pool(name="ps_o", bufs=1, space="PSUM"))
    pool_t = ctx.enter_context(tc.tile_pool(name="ps_t", bufs=2, space="PSUM"))

    ident = consts.tile([128, 128], f32)
    make_identity(nc, ident)
    identb = consts.tile([128, 128], bf16)
    nc.vector.tensor_copy(out=identb, in_=ident)

    # HAM warmup: ~3.5us of dummy matmuls at t=0 so the PE clock-gate
    # un-throttles (1.2 -> 2.4 GHz) before the real work arrives, instead of
    # ~20us into the kernel. Reads an uninitialized const tile (values are
    # irrelevant), writes a scratch PSUM slot.
    warm_src = consts.tile([128, 256], bf16, tag="warm")
    nc.gpsimd.memset(warm_src, 0.0)
    for _ in range(16):
        ps_warm = pool_t.tile([128, CH * 128], bf16, tag="t")
        nc.tensor.matmul(out=ps_warm.bitcast(f32)[:, 0:256], lhsT=identb,
                         rhs=warm_src, start=True, stop=True)

    # persistent stacked QT/KT: partitions 0:64 head A (d rows), 64:128 head B
    qt = persist.tile([128, NKB, 128], bf16, tag="qt")
    kt = persist.tile([128, NKB, 128], bf16, tag="kt")
    v1s = []
    for h in range(PAIRS_PER_CORE):
        v1 = persist.tile([128, NKB, 65], bf16, tag=f"v1{h}")
        v1s.append(v1)
        nc.gpsimd.memset(v1[:, :, 64:65], 1.0)

    def emit_vchunk(c, eng=None):
        # V rows for k-blocks [8c, 8c+8) of both heads
        nv = NKB // 4
        for h in range(PAIRS_PER_CORE):
            vnat = nat.tile([128, nv, 64], f32, tag="vnat")
            nc.sync.dma_start(
                out=vnat,
                in_=v[h].rearrange("(n p) d -> p n d", p=128)[
                    :, c * nv:(c + 1) * nv, :])
            (eng or nc.vector).tensor_copy(
                out=v1s[h][:, c * nv:(c + 1) * nv, 0:64],
                in_=vnat)

    def emit_chunk_load(src, g, eng=None):
        # DMA + cast chunk g (CH*128 rows) of both heads into one
        # [128, CH, 128] bf16 tile (head A cols 0:64, head B cols 64:128)
        natbc = nat.tile([128, CH, 128], bf16, tag="natb")
        for h in range(PAIRS_PER_CORE):
            natc = nat.tile([128, CH, 64], f32, tag=f"nat{h}")
            nc.sync.dma_start(
                out=natc,
                in_=src[h].rearrange("(n p) d -> p n d", p=128)[
                    :, g * CH:(g + 1) * CH, :])
            (eng or nc.vector).tensor_copy(
                out=natbc[:, :, h * 64:(h + 1) * 64], in_=natc)
        ps_tr = pool_t.tile([128, CH * 128], bf16, tag="t")
        return natbc, ps_tr.rearrange("p (a b) -> p a b", a=CH)

    def emit_chunk_tr(state, js):
        natbc, tr4 = state
        for j in js:
            nc.tensor.transpose(tr4[:, j, :], natbc[:, j, :], identb)

    def emit_chunk_copy(state, dst, g):
        nc.vector.tensor_copy(
            out=dst[:, g * CH:(g + 1) * CH, :], in_=state[1])

    def emit_chunk(src, dst, g, eng=None):
        st = emit_chunk_load(src, g, eng)
        emit_chunk_tr(st, range(CH))
        emit_chunk_copy(st, dst, g)

    emit_chunk(k, kt, 0, eng=nc.vector)
    emit_chunk(q, qt, 0, eng=nc.vector)
    emit_vchunk(0, eng=nc.vector)
    emit_chunk(k, kt, 1, eng=nc.vector)

    qt_f = qt.rearrange("p n d -> p (n d)")  # [128, S] q columns

    pending_epi = []
    epi_state = {}

    def emit_epi_strip(qg, oTs, strip):
        # one output strip: PE transpose + reciprocal-scale; DMA per head
        # after its 4th strip. Called with strip=0..7 spread across k-blocks.
        h, i = divmod(strip, QG // 128)
        oT = oTs[h]
        if i == 0:
            epi_state[h] = sb.tile([128, QG // 128, 64], f32, tag=f"out{h}",
                                   name=f"outsb{h}")
        out_sb = epi_state[h]
        ps_tr = pool_t.tile([128, CH * 128], bf16, tag="t")
        ps_t = ps_tr[:, 0:65]
        nc.tensor.transpose(
            ps_t, oT[:, i * 128:(i + 1) * 128], identb[0:65, 0:65])
        rcp = sb.tile([128, 1], f32, tag="rcp")
        nc.vector.reciprocal(out=rcp, in_=ps_t[:, 64:65])
        nc.scalar.mul(out=out_sb[:, i, :], in_=ps_t[:, 0:64], mul=rcp)
        if i == QG // 128 - 1:
            out_r = o[h].rearrange("(n p) d -> p n d", p=128)
            nc.sync.dma_start(
                out=out_r[:, qg * (QG // 128):(qg + 1) * (QG // 128), :],
                in_=out_sb)

    def emit_epilogue(qg, oTs):
        for strip in range(2 * (QG // 128)):
            emit_epi_strip(qg, oTs, strip)

    # ---------------- main loop --------------------------------------
    for qg in range(NQG):
        ps_oA = pool_o.tile([65, QG], f32, tag="oA")
        ps_oB = pool_o.tile([65, QG], f32, tag="oB")

        def av(prev_eT, prev_kb):
            nc.tensor.matmul(
                out=ps_oA, lhsT=v1s[0][:, prev_kb, :],
                rhs=prev_eT[:, 0:QG],
                start=(prev_kb == 0), stop=(prev_kb == NKB - 1))
            nc.tensor.matmul(
                out=ps_oB, lhsT=v1s[1][:, prev_kb, :],
                rhs=prev_eT[:, QG:2 * QG],
                start=(prev_kb == 0), stop=(prev_kb == NKB - 1))

        prev = []
        kst = qst = None
        for kb in range(NKB):
            if qg == 0:
                # stream the rest of K/V in, spread so the in-order PE queue
                # never gets a transpose burst ahead of a QK pair
                c = kb // CH + 2
                ph = kb % CH
                if c < NCHUNK:
                    if ph == 0:
                        kst = emit_chunk_load(k, c)
                        emit_chunk_tr(kst, (0, 1))
                    elif ph == 1:
                        emit_chunk_tr(kst, (2, 3))
                        emit_chunk_copy(kst, kt, c)
                if kb in (2, 10, 18):
                    emit_vchunk(kb // 8 + 1)
            if qg + 1 < NQG:
                if kb == 20:
                    qst = emit_chunk_load(q, qg + 1)
                    emit_chunk_tr(qst, (0, 1))
                elif kb == 22:
                    emit_chunk_tr(qst, (2, 3))
                    emit_chunk_copy(qst, qt, qg + 1)
            if pending_epi and 4 <= kb <= 18 and kb % 2 == 0:
                emit_epi_strip(*pending_epi[0], strip=(kb - 4) // 2)
                if kb == 18:
                    pending_epi.pop()
            ps = pool_s.tile([128, 2 * QG], f32, tag="s")
            nc.tensor.matmul(
                out=ps[:, 0:QG], lhsT=kt[0:64, kb, :],
                rhs=qt_f[0:64, qg * QG:(qg + 1) * QG],
                start=True, stop=True)
            nc.tensor.matmul(
                out=ps[:, QG:2 * QG], lhsT=kt[64:128, kb, :],
                rhs=qt_f[64:128, qg * QG:(qg + 1) * QG],
                start=True, stop=True)
            eT = pool_e.tile([128, 2 * QG], bf16, tag="exp")
            if kb % DVE_EVERY == DVE_EVERY - 1:
                nc.vector.tensor_scalar(
                    out=eT.bitcast(i16), in0=ps,
                    scalar1=SCH_A, scalar2=SCH_B,
                    op0=mybir.AluOpType.mult, op1=mybir.AluOpType.add)
            else:
                nc.scalar.activation(
                    out=eT, in_=ps,
                    func=mybir.ActivationFunctionType.Exp,
                    scale=1.0 / 16.0)
            # depth-2 software pipeline: AV for kb-2 is emitted after QK(kb)
            # so the in-order PE queue always has the next QK ahead of the
            # exp-dependent AVs -> ScalarE/DVE never starve.
            prev.append((eT, kb))
            if len(prev) > 2:
                av(*prev.pop(0))
        for pe_ in prev:
            av(*pe_)

        # drain ps_o to SBUF now (frees the banks for the next q-group);
        # the PE-transpose + normalize part is deferred into the next
        # q-group's k-loop.
        oTs = []
        for h, ps_oX in ((0, ps_oA), (1, ps_oB)):
            oT = sb.tile([65, QG], bf16, tag=f"oT{h}")
            nc.scalar.copy(out=oT, in_=ps_oX)
            oTs.append(oT)
        pending_epi.append((qg, oTs))

    emit_epilogue(*pending_epi.pop())
    ctx.close()


_CACHED = {}


def build_program():
    key = "default"
    if key in _CACHED:
        return _CACHED[key]
    nc = bacc.Bacc("TRN2", target_bir_lowering=False, debug=False,
                   num_devices=N_CORES)
    q = nc.dram_tensor("q", [PAIRS_PER_CORE, S, D], f32,
                       kind="ExternalInput").ap()
    k = nc.dram_tensor("k", [PAIRS_PER_CORE, S, D], f32,
                       kind="ExternalInput").ap()
    v = nc.dram_tensor("v", [PAIRS_PER_CORE, S, D], f32,
                       kind="ExternalInput").ap()
    o = nc.dram_tensor("o", [PAIRS_PER_CORE, S, D], f32,
                       kind="ExternalOutput").ap()
    with tile.TileContext(nc) as tc:
        build_attention(nc, tc, q, k, v, o)
    nc.compile()
    _CACHED[key] = nc
    return nc


def kernel(queries, keys, values, adj=None, **_unused):
    """Full-input attention on 8 NeuronCores. Returns [S, B, H, D] fp32."""
    queries = np.ascontiguousarray(queries, dtype=np.float32)
    keys = np.ascontiguousarray(keys, dtype=np.float32)
    values = np.ascontiguousarray(values, dtype=np.float32)

    nc = build_program()
    qf = queries.reshape(B * H, S, D)
    kf = keys.reshape(B * H, S, D)
    vf = values.reshape(B * H, S, D)
    in_maps = []
    for c in range(N_CORES):
        sl = slice(c * PAIRS_PER_CORE, (c + 1) * PAIRS_PER_CORE)
        in_maps.append({"q": qf[sl], "k": kf[sl], "v": vf[sl]})
    res = run_bass_kernel_spmd(nc, in_maps, list(range(N_CORES)))
    hout = np.empty((B * H, S, D), dtype=np.float32)
    for c in range(N_CORES):
        hout[c * PAIRS_PER_CORE:(c + 1) * PAIRS_PER_CORE] = res.results[c]["o"]
    return hout.reshape(B, H, S, D).reshape(S, B, H, D)
